# revision 1
# baseline (speedup 1.0000x reference)
"""Trainium2 Bass kernel for nn_Decoder (LSTM decoder + attention, teacher forcing).

Sharding: data-parallel over batch (64 -> 8 cores x 8 samples). The 250-step
recurrence runs locally per core; no inter-core communication.

Per-core layout (all fp32):
  - All state kept feature-major: (feature-partitions, batch-free).
  - LSTM gate matmuls: weights stationary (lhsT), batch streams (N=8).
    g1.T lives in one PSUM tile (128, 128): col 8m+b <-> gate-row 128m+p.
  - Attention energy: diag-trick MMs (lhsT = h2 (128,8) shared; rhs = masked,
    zero-padded keys (128,512) per sample) -> rows scattered at partitions
    {32j, 32j+1}; exp+rowsum on ACT; normalize rows; PE-transpose to get
    score columns; context MMs contract T with score columns as lhsT.
  - Mask folded into keys on host (zeroed beyond speech_len) => mask*energy
    is exact. Key pad cols are zero => pad energies 0 => exp=1 => Z = acc-12.
  - Vocab projection deferred: h2/ctx histories accumulated in SBUF, one
    batched matmul phase after the loop, output written vocab-major and
    transposed on host.
"""

import sys
from contextlib import ExitStack

for _p in ('/opt/trn_rl_repo', '/root/.axon_site/_ro/trn_rl_repo'):
    if _p not in sys.path:
        sys.path.insert(0, _p)

import numpy as np

import concourse.bass as bass
import concourse.tile as tile
from concourse import bacc, mybir
from concourse.bass import ts, ds
from concourse.bass_utils import run_bass_kernel_spmd
from concourse.masks import make_identity

F32 = mybir.dt.float32
AF = mybir.ActivationFunctionType
OP = mybir.AluOpType

T, B, KS, VS, H, E, VOCAB = 500, 64, 128, 128, 512, 256, 4096
NCORES, BL = 8, 8          # local batch per core
TP = 512                   # padded T (4 chunks of 128)
NTC = 4                    # number of T chunks


def build(L=250):
    nc = bacc.Bacc("TRN2", target_bir_lowering=False, debug=False,
                   num_devices=NCORES)

    # ---- DRAM I/O (per-core shapes) ----
    d_embT = nc.dram_tensor("embT", (2, 128, L * BL), F32, kind="ExternalInput").ap()
    d_w1T = nc.dram_tensor("w1T", (7, 128, 4 * H), F32, kind="ExternalInput").ap()
    d_w2T = nc.dram_tensor("w2T", (5, 128, 4 * KS), F32, kind="ExternalInput").ap()
    d_woT = nc.dram_tensor("woT", (2, 128, VOCAB), F32, kind="ExternalInput").ap()
    d_key = nc.dram_tensor("keyTm", (128, BL * TP), F32, kind="ExternalInput").ap()
    d_val = nc.dram_tensor("vT", (NTC, 128, BL * VS), F32, kind="ExternalInput").ap()
    d_v0 = nc.dram_tensor("val0T", (128, BL), F32, kind="ExternalInput").ap()
    d_bi1 = nc.dram_tensor("b_ih1", (1, 4 * H), F32, kind="ExternalInput").ap()
    d_bh1 = nc.dram_tensor("b_hh1", (1, 4 * H), F32, kind="ExternalInput").ap()
    d_bi2 = nc.dram_tensor("b_ih2", (1, 4 * KS), F32, kind="ExternalInput").ap()
    d_bh2 = nc.dram_tensor("b_hh2", (1, 4 * KS), F32, kind="ExternalInput").ap()
    d_bo = nc.dram_tensor("b_outS", (128, VOCAB // 128), F32, kind="ExternalInput").ap()
    d_out = nc.dram_tensor("predT", (VOCAB // 128, 128, L * BL), F32,
                           kind="ExternalOutput").ap()

    with tile.TileContext(nc) as tc, ExitStack() as ctx:
        singles = ctx.enter_context(tc.tile_pool(name="singles", bufs=1))

        # ---- SBUF resident tensors ----
        w1Ts = singles.tile([128, 7, 4 * H], F32)       # 7.3 MB
        w2Ts = singles.tile([128, 5, 4 * KS], F32)      # 1.3 MB
        woTs = singles.tile([128, 2, VOCAB], F32)       # 4.2 MB
        embTs = singles.tile([128, 2, L * BL], F32)     # 2.0 MB
        keyTs = singles.tile([128, BL * TP], F32)       # 2.0 MB
        vTs = singles.tile([128, NTC, BL, VS], F32)     # 2.1 MB
        histH = singles.tile([128, L * BL], F32)        # 1.0 MB
        histC = singles.tile([128, L * BL], F32)        # 1.0 MB
        b1row = singles.tile([1, 4 * H], F32)
        b2row = singles.tile([1, 4 * KS], F32)
        bo_s = singles.tile([128, VOCAB // 128], F32)
        ones8 = singles.tile([1, BL], F32)
        ident = singles.tile([128, 128], F32)

        # states
        h1 = singles.tile([128, 32], F32)   # h1.T: [p, 8m+b], h=128m+p
        c1 = singles.tile([128, 32], F32)
        h2 = singles.tile([128, BL], F32)   # h2.T
        c2 = singles.tile([128, BL], F32)
        ctxT = singles.tile([128, BL], F32)  # context.T

        tmp_b1 = singles.tile([1, 4 * H], F32)
        tmp_b2 = singles.tile([1, 4 * KS], F32)

        # ---- prologue: loads ----
        for kc in range(7):
            nc.sync.dma_start(w1Ts[:, kc, :], d_w1T[kc])
        for kc in range(5):
            nc.sync.dma_start(w2Ts[:, kc, :], d_w2T[kc])
        for kc in range(2):
            nc.sync.dma_start(woTs[:, kc, :], d_woT[kc])
            nc.sync.dma_start(embTs[:, kc, :], d_embT[kc])
        nc.sync.dma_start(keyTs[:], d_key[:])
        for tcn in range(NTC):
            nc.sync.dma_start(vTs[:, tcn, :, :], d_val[tcn])
        nc.sync.dma_start(ctxT[:], d_v0[:])
        nc.sync.dma_start(b1row[:], d_bi1[:])
        nc.sync.dma_start(tmp_b1[:], d_bh1[:])
        nc.sync.dma_start(b2row[:], d_bi2[:])
        nc.sync.dma_start(tmp_b2[:], d_bh2[:])
        nc.sync.dma_start(bo_s[:], d_bo[:])

        nc.vector.tensor_add(b1row[:], b1row[:], tmp_b1[:])
        nc.vector.tensor_add(b2row[:], b2row[:], tmp_b2[:])
        nc.vector.memset(ones8[:], 1.0)
        make_identity(nc, ident[:])
        nc.vector.memset(h1[:], 0.0)
        nc.vector.memset(c1[:], 0.0)
        nc.vector.memset(h2[:], 0.0)
        nc.vector.memset(c2[:], 0.0)

        # ---- PSUM pools (scoped to the recurrence loop) ----
        loop_ctx = ctx.enter_context(ExitStack())
        ppool = loop_ctx.enter_context(tc.tile_pool(name="ppool", bufs=1, space="PSUM"))
        trpool = loop_ctx.enter_context(tc.tile_pool(name="trpool", bufs=2, space="PSUM"))
        # ---- SBUF temp pool ----
        temps = loop_ctx.enter_context(tc.tile_pool(name="temps", bufs=2))

        def step(t):
            # ===== LSTM1: g1.T in PSUM (128,128); col 8m+b = gate-row 128m+p
            pg1 = ppool.tile([128, 128], F32, tag="pg1")
            for m in range(16):
                o = pg1[:, m * BL:(m + 1) * BL]
                for kc in range(7):
                    if kc < 2:
                        rhs = embTs[:, kc, ds(t * BL, BL)]
                    elif kc == 2:
                        rhs = ctxT[:]
                    else:
                        rhs = h1[:, (kc - 3) * BL:(kc - 2) * BL]
                    nc.tensor.matmul(o, w1Ts[:, kc, m * 128:(m + 1) * 128], rhs,
                                     start=(kc == 0), stop=False)
                nc.tensor.matmul(o, b1row[:, m * 128:(m + 1) * 128], ones8[:],
                                 start=False, stop=True)
            # gates1: i cols 0:32, f 32:64, g 64:96, o 96:128
            s_if = temps.tile([128, 64], F32, tag="s_if")
            s_g = temps.tile([128, 32], F32, tag="s_g")
            s_o = temps.tile([128, 32], F32, tag="s_o")
            nc.scalar.activation(s_if[:], pg1[:, 0:64], AF.Sigmoid)
            nc.scalar.activation(s_g[:], pg1[:, 64:96], AF.Tanh)
            nc.scalar.activation(s_o[:], pg1[:, 96:128], AF.Sigmoid)
            m1 = temps.tile([128, 32], F32, tag="m1")
            nc.vector.tensor_mul(m1[:], s_if[:, 0:32], s_g[:])
            nc.vector.tensor_mul(c1[:], s_if[:, 32:64], c1[:])
            nc.vector.tensor_add(c1[:], c1[:], m1[:])
            tc1 = temps.tile([128, 32], F32, tag="tc1")
            nc.scalar.activation(tc1[:], c1[:], AF.Tanh)
            nc.vector.tensor_mul(h1[:], s_o[:], tc1[:])

            # ===== LSTM2: g2.T in PSUM (128,32); col 8m+b = gate-row 128m+p
            pg2 = ppool.tile([128, 32], F32, tag="pg2")
            for m in range(4):
                o = pg2[:, m * BL:(m + 1) * BL]
                for kc in range(5):
                    rhs = h1[:, kc * BL:(kc + 1) * BL] if kc < 4 else h2[:]
                    nc.tensor.matmul(o, w2Ts[:, kc, m * 128:(m + 1) * 128], rhs,
                                     start=(kc == 0), stop=False)
                nc.tensor.matmul(o, b2row[:, m * 128:(m + 1) * 128], ones8[:],
                                 start=False, stop=True)
            s_if2 = temps.tile([128, 16], F32, tag="s_if2")
            s_g2 = temps.tile([128, 8], F32, tag="s_g2")
            s_o2 = temps.tile([128, 8], F32, tag="s_o2")
            nc.scalar.activation(s_if2[:], pg2[:, 0:16], AF.Sigmoid)
            nc.scalar.activation(s_g2[:], pg2[:, 16:24], AF.Tanh)
            nc.scalar.activation(s_o2[:], pg2[:, 24:32], AF.Sigmoid)
            m12 = temps.tile([128, 8], F32, tag="m12")
            nc.vector.tensor_mul(m12[:], s_if2[:, 0:8], s_g2[:])
            nc.vector.tensor_mul(c2[:], s_if2[:, 8:16], c2[:])
            nc.vector.tensor_add(c2[:], c2[:], m12[:])
            tc2 = temps.tile([128, 8], F32, tag="tc2")
            nc.scalar.activation(tc2[:], c2[:], AF.Tanh)
            nc.vector.tensor_mul(h2[:], s_o2[:], tc2[:])
            nc.gpsimd.tensor_copy(histH[:, ds(t * BL, BL)], h2[:])

            # ===== attention =====
            # energy: rows at partition 32j+h for sample b=2j+h, half h
            pE = ppool.tile([104, 2 * TP], F32, tag="pE")
            for j in range(4):
                for hh in range(2):
                    b = 2 * j + hh
                    nc.tensor.matmul(
                        pE[32 * j:32 * j + 8, hh * TP:(hh + 1) * TP],
                        h2[:], keyTs[:, b * TP:(b + 1) * TP],
                        start=True, stop=True, tile_position=(0, 32 * j))
            # exp + row sums
            expS = temps.tile([104, 2 * TP], F32, tag="expS")
            zacc = temps.tile([104, 2], F32, tag="zacc")
            for hh in range(2):
                nc.scalar.activation(expS[:, hh * TP:(hh + 1) * TP],
                                     pE[:, hh * TP:(hh + 1) * TP], AF.Exp,
                                     accum_out=zacc[:, hh:hh + 1])
            # Z = acc - (TP - T) pad ones; score rows = exp * (1/Z)
            zr = temps.tile([104, 2], F32, tag="zr")
            nc.vector.tensor_scalar_add(zr[:], zacc[:], -float(TP - T))
            nc.vector.reciprocal(zr[:], zr[:])
            scoreS = temps.tile([104, 2 * TP], F32, tag="scoreS")
            for hh in range(2):
                nc.vector.tensor_scalar_mul(scoreS[:, hh * TP:(hh + 1) * TP],
                                            expS[:, hh * TP:(hh + 1) * TP],
                                            zr[:, hh:hh + 1])
            # transpose scores -> columns; extract valid cols {34j+h}
            scT = temps.tile([128, NTC, BL], F32, tag="scT")
            for hh in range(2):
                for tcn in range(NTC):
                    ptr = trpool.tile([128, 104], F32, tag="ptr")
                    nc.tensor.transpose(
                        ptr[:], scoreS[0:104, hh * TP + tcn * 128: hh * TP + (tcn + 1) * 128],
                        ident[0:104, 0:104])
                    nc.vector.tensor_copy(scT[:, tcn, hh::2], ptr[:, hh::34])
            # context: ctxU rows at partition 32j, half hh in cols 128hh:+128
            pCtx = ppool.tile([97, 2 * VS], F32, tag="pCtx")
            for j in range(4):
                for hh in range(2):
                    b = 2 * j + hh
                    for tcn in range(NTC):
                        nc.tensor.matmul(
                            pCtx[32 * j:32 * j + 1, hh * VS:(hh + 1) * VS],
                            scT[:, tcn, b:b + 1], vTs[:, tcn, b, :],
                            start=(tcn == 0), stop=(tcn == NTC - 1),
                            tile_position=(0, 32 * j))
            ctxUS = temps.tile([97, 2 * VS], F32, tag="ctxUS")
            nc.vector.tensor_copy(ctxUS[:], pCtx[:])
            for hh in range(2):
                ptc = trpool.tile([128, 97], F32, tag="ptr")
                nc.tensor.transpose(ptc[:], ctxUS[0:97, hh * VS:(hh + 1) * VS],
                                    ident[0:97, 0:97])
                nc.vector.tensor_copy(ctxT[:, hh::2], ptc[:, 0::32])
            nc.gpsimd.tensor_copy(histC[:, ds(t * BL, BL)], ctxT[:])

        with tc.For_i(0, L) as t:
            step(t)
        loop_ctx.close()

        # ===== deferred vocab projection =====
        NB = 4
        nblk = (L * BL) // NB
        with tc.tile_pool(name="projp", bufs=2, space="PSUM") as projp, \
             tc.tile_pool(name="projs", bufs=3) as projs:
            for vc in range(VOCAB // 128):
                for nb in range(NB):
                    pp = projp.tile([128, nblk], F32, tag="pp")
                    sl = ds(nb * nblk, nblk)
                    nc.tensor.matmul(pp[:], woTs[:, 0, vc * 128:(vc + 1) * 128],
                                     histH[:, sl], start=True, stop=False)
                    nc.tensor.matmul(pp[:], woTs[:, 1, vc * 128:(vc + 1) * 128],
                                     histC[:, sl], start=False, stop=True)
                    ob = projs.tile([128, nblk], F32, tag="ob")
                    nc.vector.tensor_scalar_add(ob[:], pp[:], bo_s[:, vc:vc + 1])
                    nc.sync.dma_start(d_out[vc][:, sl], ob[:])

    nc.compile()
    return nc


_CACHE = {}


def _get_nc(L):
    if L not in _CACHE:
        _CACHE[L] = build(L)
    return _CACHE[L]


def _prep_inputs(key, values, speech_len, text, embedding,
                 w_ih1, b_ih1, w_hh1, b_hh1,
                 w_ih2, b_ih2, w_hh2, b_hh2,
                 w_out, b_out, L):
    f = np.float32
    key = np.asarray(key, f)
    values = np.asarray(values, f)
    speech_len = np.asarray(speech_len)
    text = np.asarray(text)
    embedding = np.asarray(embedding, f)

    # shared (replicated) tensors
    w1T = np.ascontiguousarray(
        np.concatenate([np.asarray(w_ih1, f), np.asarray(w_hh1, f)], axis=1)
        .T.reshape(7, 128, 4 * H))
    w2T = np.ascontiguousarray(
        np.concatenate([np.asarray(w_ih2, f), np.asarray(w_hh2, f)], axis=1)
        .T.reshape(5, 128, 4 * KS))
    woT = np.ascontiguousarray(np.asarray(w_out, f).T.reshape(2, 128, VOCAB))
    b_outS = np.ascontiguousarray(np.asarray(b_out, f).reshape(VOCAB // 128, 128).T)
    shared = {
        "w1T": w1T, "w2T": w2T, "woT": woT,
        "b_ih1": np.asarray(b_ih1, f).reshape(1, -1),
        "b_hh1": np.asarray(b_hh1, f).reshape(1, -1),
        "b_ih2": np.asarray(b_ih2, f).reshape(1, -1),
        "b_hh2": np.asarray(b_hh2, f).reshape(1, -1),
        "b_outS": b_outS,
    }

    # teacher-forcing tokens and embeddings (host gather)
    tokens = np.concatenate(
        [np.zeros((B, 1), text.dtype), text[:, :L - 1]], axis=1)  # (B, L)
    embeds = embedding[tokens]  # (B, L, E)

    mask = (np.arange(T)[:, None] < np.asarray(speech_len)[None, :])  # (T, B)

    in_maps = []
    for c in range(NCORES):
        bs = slice(c * BL, (c + 1) * BL)
        embT = np.ascontiguousarray(
            embeds[bs].transpose(2, 1, 0).reshape(2, 128, L * BL))  # [e,(t,b)]
        km = key[:, bs, :] * mask[:, bs, None].astype(f)  # (T, BL, KS)
        kT = np.zeros((128, BL, TP), f)
        kT[:, :, :T] = km.transpose(2, 1, 0)
        v = np.zeros((TP, BL, VS), f)
        v[:T] = values[:, bs, :]
        vT = np.ascontiguousarray(v.reshape(NTC, 128, BL * VS))
        in_maps.append(dict(
            embT=embT,
            keyTm=np.ascontiguousarray(kT.reshape(128, BL * TP)),
            vT=vT,
            val0T=np.ascontiguousarray(values[0, bs, :].T),
            **shared))
    return in_maps


def kernel(key, values, speech_len, text, embedding,
           w_ih1, b_ih1, w_hh1, b_hh1,
           w_ih2, b_ih2, w_hh2, b_hh2,
           w_out, b_out, _L=250, _trace=False, _tmpdir=None):
    L = _L
    nc = _get_nc(L)
    in_maps = _prep_inputs(key, values, speech_len, text, embedding,
                           w_ih1, b_ih1, w_hh1, b_hh1,
                           w_ih2, b_ih2, w_hh2, b_hh2, w_out, b_out, L)
    kw = {}
    if _trace:
        kw = dict(trace=True, tmpdir=_tmpdir)
    res = run_bass_kernel_spmd(nc, in_maps, core_ids=list(range(NCORES)), **kw)
    kernel._last = res
    out = np.empty((B, L, VOCAB), np.float32)
    for c in range(NCORES):
        p = res.results[c]["predT"]  # (32, 128, L*BL)
        out[c * BL:(c + 1) * BL] = (
            p.reshape(VOCAB // 128, 128, L, BL).transpose(3, 2, 0, 1)
            .reshape(BL, L, VOCAB))
    return out



# revision 4
# speedup vs baseline: 2.6793x; 2.6793x over previous
"""Trainium2 Bass kernel for nn_Decoder (LSTM decoder + attention, teacher forcing).

Sharding: data-parallel over batch (64 -> 8 cores x 8 samples). The 250-step
recurrence runs locally per core; no inter-core communication.

v2: flipped-matmul bf16 design. The per-step matmuls keep the tiny batch-8
activations STATIONARY (lhsT) and stream the weights as bf16 moving operand,
so the big weight matrices cross the PE once per step at 1 cycle/row instead
of being re-loaded as 128x128 LDWEIGHTS tiles (which dominated the fp32
baseline). Gates land batch-major in PSUM ([8, 4H]); biases are folded in as
ones-row matmuls. Attention context is accumulated directly in transposed
layout (stationary = value chunks, moving = score columns), ready to be next
step's LSTM1 input. Vocab projection is deferred and batched after the loop.
"""

import sys
from contextlib import ExitStack

for _p in ('/opt/trn_rl_repo', '/root/.axon_site/_ro/trn_rl_repo'):
    if _p not in sys.path:
        sys.path.insert(0, _p)

import numpy as np
import ml_dtypes

import concourse.bass as bass
import concourse.tile as tile
from concourse import bacc, mybir
from concourse.bass import ts, ds
from concourse.bass_utils import run_bass_kernel_spmd
from concourse.masks import make_identity

F32 = mybir.dt.float32
BF16 = mybir.dt.bfloat16
AF = mybir.ActivationFunctionType
OP = mybir.AluOpType
BFNP = ml_dtypes.bfloat16

T, B, KS, VS, H, E, VOCAB = 500, 64, 128, 128, 512, 256, 4096
NCORES, BL = 8, 8          # local batch per core
TP = 512                   # padded T (4 chunks of 128)
NTC = 4                    # number of T chunks
G1 = 4 * H                 # 2048 LSTM1 gate cols
G2 = 4 * KS                # 512 LSTM2 gate cols


def build(L=250):
    nc = bacc.Bacc("TRN2", target_bir_lowering=False, debug=False,
                   num_devices=NCORES)

    # ---- DRAM I/O (per-core shapes) ----
    d_embT = nc.dram_tensor("embT", (2, 128, (L + 1) * BL), BF16, kind="ExternalInput").ap()
    d_w1T = nc.dram_tensor("w1T", (7, 128, G1), BF16, kind="ExternalInput").ap()
    d_w2T = nc.dram_tensor("w2T", (5, 128, G2), BF16, kind="ExternalInput").ap()
    d_woT = nc.dram_tensor("woT", (2, 128, VOCAB), BF16, kind="ExternalInput").ap()
    d_key = nc.dram_tensor("keyTm", (128, BL * TP), BF16, kind="ExternalInput").ap()
    d_val = nc.dram_tensor("vT", (NTC, 128, BL * VS), BF16, kind="ExternalInput").ap()
    d_v0 = nc.dram_tensor("val0T", (128, BL), BF16, kind="ExternalInput").ap()
    d_b1 = nc.dram_tensor("b1row", (1, G1), BF16, kind="ExternalInput").ap()
    d_b2 = nc.dram_tensor("b2row", (1, G2), BF16, kind="ExternalInput").ap()
    d_bo = nc.dram_tensor("b_outS", (128, VOCAB // 128), F32, kind="ExternalInput").ap()
    d_out = nc.dram_tensor("predT", (VOCAB // 128, 128, L * BL), F32,
                           kind="ExternalOutput").ap()

    with tile.TileContext(nc) as tc, ExitStack() as ctx:
        singles = ctx.enter_context(tc.tile_pool(name="singles", bufs=1))

        # ---- SBUF resident tensors (bf16 operands) ----
        w1Ts = singles.tile([128, 7, G1], BF16)          # 3.5 MB
        w2Ts = singles.tile([128, 5, G2], BF16)
        woTs = singles.tile([128, 2, VOCAB], BF16)
        embTs = singles.tile([128, 2, (L + 1) * BL], BF16)
        keyTs = singles.tile([128, BL * TP], BF16)
        vTs = singles.tile([128, NTC, BL, VS], BF16)
        histH = singles.tile([128, L * BL], BF16)
        histC = singles.tile([128, L * BL], BF16)
        b1row = singles.tile([1, G1], BF16)
        b2row = singles.tile([1, G2], BF16)
        bo_s = singles.tile([128, VOCAB // 128], F32)
        ones1 = singles.tile([1, BL], BF16)
        identf = singles.tile([128, 128], F32)

        # recurrent state
        h1T = singles.tile([128, 4 * BL], BF16)   # h1.T: [h-chunk part, 8b]
        h2T = singles.tile([128, BL], BF16)
        ctxT = singles.tile([128, BL], BF16)
        c1 = singles.tile([BL, H], F32)           # batch-major cells
        c2 = singles.tile([BL, KS], F32)
        embX = singles.tile([128, 2, BL], BF16)   # this step's emb (lhsT fixed)

        # ---- prologue loads ----
        for kc in range(7):
            nc.sync.dma_start(w1Ts[:, kc, :], d_w1T[kc])
        for kc in range(5):
            nc.sync.dma_start(w2Ts[:, kc, :], d_w2T[kc])
        for kc in range(2):
            nc.sync.dma_start(woTs[:, kc, :], d_woT[kc])
            nc.sync.dma_start(embTs[:, kc, 0:L * BL], d_embT[kc][:, 0:L * BL])
            nc.vector.memset(embTs[:, kc, L * BL:(L + 1) * BL], 0.0)
        nc.sync.dma_start(keyTs[:], d_key[:])
        for tcn in range(NTC):
            nc.sync.dma_start(vTs[:, tcn, :, :], d_val[tcn])
        nc.sync.dma_start(ctxT[:], d_v0[:])
        nc.sync.dma_start(b1row[:], d_b1[:])
        nc.sync.dma_start(b2row[:], d_b2[:])
        nc.sync.dma_start(bo_s[:], d_bo[:])

        nc.vector.memset(ones1[:], 1.0)
        make_identity(nc, identf[:])
        nc.vector.memset(h1T[:], 0.0)
        nc.vector.memset(h2T[:], 0.0)
        nc.vector.memset(c1[:], 0.0)
        nc.vector.memset(c2[:], 0.0)

        loop_ctx = ctx.enter_context(ExitStack())
        # PSUM (bank-granular): pg1 4 + pE 2 + pg2 1 + shared small bank 1 = 8
        ppool = loop_ctx.enter_context(tc.tile_pool(name="ppool", bufs=1, space="PSUM"))
        temps = loop_ctx.enter_context(tc.tile_pool(name="temps", bufs=2))

        pg1 = ppool.tile([BL, G1], F32, tag="pg1")
        pg2 = ppool.tile([BL, G2], F32, tag="pg2")
        pE = ppool.tile([104, 2 * TP], F32, tag="pE")
        psmall = ppool.tile([128, 512], F32, tag="psmall")
        trH = psmall[:, 0:32]
        trH2 = psmall[:, 32:40]
        ptrs = [psmall[:, 40:144], psmall[:, 144:248]]
        pCtxT = psmall[:, 248:256]

        def step(t):
            # ---- this step's embedding columns (lhsT must have static offset)
            nc.vector.tensor_copy(embX[:], embTs[:, :, ds(t * BL, BL)])

            # ===== LSTM1: pg1[8, 2048] batch-major; bank b = gate-col block
            for bk in range(4):
                o = pg1[:, bk * 512:(bk + 1) * 512]
                w = w1Ts[:, :, bk * 512:(bk + 1) * 512]
                nc.tensor.matmul(o, embX[:, 0, :], w[:, 0, :], start=True, stop=False)
                nc.tensor.matmul(o, embX[:, 1, :], w[:, 1, :], start=False, stop=False)
                nc.tensor.matmul(o, ones1[:], b1row[:, bk * 512:(bk + 1) * 512],
                                 start=False, stop=False)
                for hc in range(4):
                    nc.tensor.matmul(o, h1T[:, hc * BL:(hc + 1) * BL], w[:, 3 + hc, :],
                                     start=False, stop=False)
                nc.tensor.matmul(o, ctxT[:], w[:, 2, :], start=False, stop=True)

            # gates1: cols 0:512 i, 512:1024 f, 1024:1536 g, 1536:2048 o
            sif = temps.tile([BL, 1024], F32, tag="sif")
            sg = temps.tile([BL, 512], F32, tag="sg")
            so = temps.tile([BL, 512], F32, tag="so")
            nc.scalar.activation(sif[:], pg1[:, 0:1024], AF.Sigmoid)
            nc.scalar.activation(sg[:], pg1[:, 1024:1536], AF.Tanh)
            nc.scalar.activation(so[:], pg1[:, 1536:2048], AF.Sigmoid)
            m1 = temps.tile([BL, 512], F32, tag="m1")
            nc.vector.tensor_mul(m1[:], sif[:, 0:512], sg[:])
            nc.vector.tensor_mul(c1[:], sif[:, 512:1024], c1[:])
            nc.vector.tensor_add(c1[:], c1[:], m1[:])
            tc1 = temps.tile([BL, 512], F32, tag="tc1")
            nc.scalar.activation(tc1[:], c1[:], AF.Tanh)
            h1b = temps.tile([BL, 512], F32, tag="h1b")
            nc.vector.tensor_mul(h1b[:], so[:], tc1[:])

            # h1T <- transpose(h1b)
            for hc in range(4):
                nc.tensor.transpose(trH[:, hc * BL:(hc + 1) * BL],
                                    h1b[:, hc * 128:(hc + 1) * 128],
                                    identf[0:BL, 0:BL])
            nc.vector.tensor_copy(h1T[:], trH[:])

            # ===== LSTM2: pg2[8, 512]
            for hc in range(4):
                nc.tensor.matmul(pg2[:], h1T[:, hc * BL:(hc + 1) * BL], w2Ts[:, hc, :],
                                 start=(hc == 0), stop=False)
            nc.tensor.matmul(pg2[:], ones1[:], b2row[:], start=False, stop=False)
            nc.tensor.matmul(pg2[:], h2T[:], w2Ts[:, 4, :], start=False, stop=True)

            sif2 = temps.tile([BL, 256], F32, tag="sif2")
            sg2 = temps.tile([BL, 128], F32, tag="sg2")
            so2 = temps.tile([BL, 128], F32, tag="so2")
            nc.scalar.activation(sif2[:], pg2[:, 0:256], AF.Sigmoid)
            nc.scalar.activation(sg2[:], pg2[:, 256:384], AF.Tanh)
            nc.scalar.activation(so2[:], pg2[:, 384:512], AF.Sigmoid)
            m2 = temps.tile([BL, 128], F32, tag="m2")
            nc.vector.tensor_mul(m2[:], sif2[:, 0:128], sg2[:])
            nc.vector.tensor_mul(c2[:], sif2[:, 128:256], c2[:])
            nc.vector.tensor_add(c2[:], c2[:], m2[:])
            tc2 = temps.tile([BL, 128], F32, tag="tc2")
            nc.scalar.activation(tc2[:], c2[:], AF.Tanh)
            h2b = temps.tile([BL, 128], F32, tag="h2b")
            nc.vector.tensor_mul(h2b[:], so2[:], tc2[:])

            nc.tensor.transpose(trH2[:], h2b[:], identf[0:BL, 0:BL])
            nc.vector.tensor_copy(h2T[:], trH2[:])
            nc.gpsimd.tensor_copy(histH[:, ds(t * BL, BL)], h2T[:])

            # ===== attention =====
            # energy rows at partition 34j+hh for sample b=2j+hh
            for j in range(4):
                for hh in range(2):
                    b = 2 * j + hh
                    nc.tensor.matmul(
                        pE[32 * j:32 * j + 8, hh * TP:(hh + 1) * TP],
                        h2T[:], keyTs[:, b * TP:(b + 1) * TP],
                        start=True, stop=True, tile_position=(0, 32 * j))
            # exp + row sums; Z = acc - (TP - T) pad ones
            expS = temps.tile([104, 2 * TP], F32, tag="expS")
            zacc = temps.tile([104, 2], F32, tag="zacc")
            for hh in range(2):
                nc.scalar.activation(expS[:, hh * TP:(hh + 1) * TP],
                                     pE[:, hh * TP:(hh + 1) * TP], AF.Exp,
                                     accum_out=zacc[:, hh:hh + 1])
            zr = temps.tile([104, 2], F32, tag="zr")
            nc.vector.tensor_scalar_add(zr[:], zacc[:], -float(TP - T))
            nc.vector.reciprocal(zr[:], zr[:])
            for hh in range(2):
                nc.vector.tensor_scalar_mul(expS[:, hh * TP:(hh + 1) * TP],
                                            expS[:, hh * TP:(hh + 1) * TP],
                                            zr[:, hh:hh + 1])
            # transpose normalized scores -> columns; valid cols {34j+hh}
            scT = temps.tile([128, NTC, BL], BF16, tag="scT")
            for hh in range(2):
                for tcn in range(NTC):
                    ptr = ptrs[(hh * NTC + tcn) % 2]
                    nc.tensor.transpose(
                        ptr[0:128, 0:104],
                        expS[0:104, hh * TP + tcn * 128: hh * TP + (tcn + 1) * 128],
                        identf[0:104, 0:104])
                    nc.vector.tensor_copy(scT[:, tcn, hh::2], ptr[:, hh::34])
            # context directly transposed: stationary = V chunk, moving = score col
            for b in range(BL):
                for tcn in range(NTC):
                    nc.tensor.matmul(pCtxT[:, b:b + 1], vTs[:, tcn, b, :],
                                     scT[:, tcn, b:b + 1],
                                     start=(tcn == 0), stop=(tcn == NTC - 1))
            nc.vector.tensor_copy(ctxT[:], pCtxT[:])
            nc.gpsimd.tensor_copy(histC[:, ds(t * BL, BL)], ctxT[:])

        with tc.For_i(0, L) as t:
            step(t)
        loop_ctx.close()

        # ===== deferred vocab projection =====
        NB = 4
        nblk = (L * BL) // NB
        with tc.tile_pool(name="projp", bufs=2, space="PSUM") as projp, \
             tc.tile_pool(name="projs", bufs=3) as projs:
            for vc in range(VOCAB // 128):
                for nb in range(NB):
                    pp = projp.tile([128, nblk], F32, tag="pp")
                    sl = ds(nb * nblk, nblk)
                    nc.tensor.matmul(pp[:], woTs[:, 0, vc * 128:(vc + 1) * 128],
                                     histH[:, sl], start=True, stop=False)
                    nc.tensor.matmul(pp[:], woTs[:, 1, vc * 128:(vc + 1) * 128],
                                     histC[:, sl], start=False, stop=True)
                    ob = projs.tile([128, nblk], F32, tag="ob")
                    nc.vector.tensor_scalar_add(ob[:], pp[:], bo_s[:, vc:vc + 1])
                    nc.sync.dma_start(d_out[vc][:, sl], ob[:])

    nc.compile()
    return nc


_CACHE = {}


def _get_nc(L):
    if L not in _CACHE:
        _CACHE[L] = build(L)
    return _CACHE[L]


def _prep_inputs(key, values, speech_len, text, embedding,
                 w_ih1, b_ih1, w_hh1, b_hh1,
                 w_ih2, b_ih2, w_hh2, b_hh2,
                 w_out, b_out, L):
    f = np.float32
    key = np.asarray(key, f)
    values = np.asarray(values, f)
    speech_len = np.asarray(speech_len)
    text = np.asarray(text)
    embedding = np.asarray(embedding, f)

    w1T = np.ascontiguousarray(
        np.concatenate([np.asarray(w_ih1, f), np.asarray(w_hh1, f)], axis=1)
        .T.reshape(7, 128, G1)).astype(BFNP)
    w2T = np.ascontiguousarray(
        np.concatenate([np.asarray(w_ih2, f), np.asarray(w_hh2, f)], axis=1)
        .T.reshape(5, 128, G2)).astype(BFNP)
    woT = np.ascontiguousarray(np.asarray(w_out, f).T.reshape(2, 128, VOCAB)).astype(BFNP)
    b_outS = np.ascontiguousarray(np.asarray(b_out, f).reshape(VOCAB // 128, 128).T)
    shared = {
        "w1T": w1T, "w2T": w2T, "woT": woT,
        "b1row": (np.asarray(b_ih1, f) + np.asarray(b_hh1, f)).reshape(1, -1).astype(BFNP),
        "b2row": (np.asarray(b_ih2, f) + np.asarray(b_hh2, f)).reshape(1, -1).astype(BFNP),
        "b_outS": b_outS,
    }

    tokens = np.concatenate(
        [np.zeros((B, 1), text.dtype), text[:, :L - 1]], axis=1)  # (B, L)
    embeds = embedding[tokens]  # (B, L, E)

    mask = (np.arange(T)[:, None] < np.asarray(speech_len)[None, :])  # (T, B)

    in_maps = []
    for c in range(NCORES):
        bs = slice(c * BL, (c + 1) * BL)
        embT = np.zeros((2, 128, (L + 1) * BL), BFNP)
        embT[:, :, :L * BL] = embeds[bs].transpose(2, 1, 0).reshape(
            2, 128, L * BL).astype(BFNP)
        km = key[:, bs, :] * mask[:, bs, None].astype(f)  # (T, BL, KS)
        kT = np.zeros((128, BL, TP), f)
        kT[:, :, :T] = km.transpose(2, 1, 0)
        v = np.zeros((TP, BL, VS), f)
        v[:T] = values[:, bs, :]
        vT = np.ascontiguousarray(v.reshape(NTC, 128, BL * VS)).astype(BFNP)
        in_maps.append(dict(
            embT=embT,
            keyTm=np.ascontiguousarray(kT.reshape(128, BL * TP)).astype(BFNP),
            vT=vT,
            val0T=np.ascontiguousarray(values[0, bs, :].T).astype(BFNP),
            **shared))
    return in_maps


def kernel(key, values, speech_len, text, embedding,
           w_ih1, b_ih1, w_hh1, b_hh1,
           w_ih2, b_ih2, w_hh2, b_hh2,
           w_out, b_out, _L=250, _trace=False, _tmpdir=None):
    L = _L
    nc = _get_nc(L)
    in_maps = _prep_inputs(key, values, speech_len, text, embedding,
                           w_ih1, b_ih1, w_hh1, b_hh1,
                           w_ih2, b_ih2, w_hh2, b_hh2, w_out, b_out, L)
    kw = {}
    if _trace:
        kw = dict(trace=True, tmpdir=_tmpdir)
    res = run_bass_kernel_spmd(nc, in_maps, core_ids=list(range(NCORES)), **kw)
    kernel._last = res
    out = np.empty((B, L, VOCAB), np.float32)
    for c in range(NCORES):
        p = res.results[c]["predT"]  # (32, 128, L*BL)
        out[c * BL:(c + 1) * BL] = (
            p.reshape(VOCAB // 128, 128, L, BL).transpose(3, 2, 0, 1)
            .reshape(BL, L, VOCAB))
    return out
